# revision 1
# baseline (speedup 1.0000x reference)
"""Trainium2 Bass kernel for a fused multi-head attention block.

Reference computation (B=4, T=2048, D=1152, H=8, HD=144, full rotary):
    q,k,v = x@Wq.T, x@Wk.T, x@Wv.T   (per head)
    q,k   = rope(q, k, cos, sin)
    o     = softmax(q k^T / sqrt(HD)) v
    out   = o @ Wo.T

Sharding (8 cores): core c = (batch b = c//2, head-group hg = c%2).
Each core computes 4 heads of one batch and a partial output
out_part = o_local @ Wo[:, hg_cols].T ; host sums the two partials per batch.

Per-core layout decisions:
  * Host passes x transposed (xT [D, T]) and weights pre-transposed so that
    every matmul contraction sits on the partition axis.
  * q/k head dims are padded 144 -> 160 and reordered on the host into
    [h0:0-127 | h1:0-127 | h2:0-127 | h3:0-127 | b-block 4x(16 real + 16 zero)]
    so that per-head tiles stay 128/32-partition aligned on chip.
  * Scores are computed transposed (S^T [keys, q]) so the PV matmul needs no
    transpose, and the softmax denominator comes free by appending a ones
    column to v (o_psum[:, 144] = sum(exp(S))).
  * exp() has no max-subtraction: scores*scale have std ~0.7, |S|<6, safely
    inside fp32/bf16 exp range.
  * dtypes: projections/final in f32r (fp32 bits, fast PE path), attention
    matmuls in bf16, all accumulation fp32 in PSUM.
"""

import numpy as np

B, T, D, H = 4, 2048, 1152, 8
HL = 4              # heads per core
HD = 144            # head dim
EP = 640            # padded q/k projection width: 4*128 + 128 (4x(16+16pad))
DV = HL * HD        # 576, v/o width
NT = T // 128       # 16 t-tiles
KC = D // 128       # 9 contraction chunks
SCALE = float(HD) ** -0.5
NCORES = 8

_NC_CACHE = {}
GSZ = 4  # score key-tiles per burst group (1 = no b-burst packing)


def _build(debug=False, gsz=None):
    gsz = GSZ if gsz is None else gsz
    import concourse.bacc as bacc
    import concourse.mybir as mybir
    from concourse.tile import TileContext

    dt = mybir.dt
    f32, f32r, bf16 = dt.float32, dt.float32r, dt.bfloat16
    AF = mybir.ActivationFunctionType

    nc = bacc.Bacc(
        "TRN2",
        target_bir_lowering=False,
        debug=debug,
        enable_asserts=False,
        num_devices=NCORES,
    )

    xT = nc.declare_dram_parameter("xT", [D, T], bf16, isOutput=False)
    wqT = nc.declare_dram_parameter("wqT", [D, EP], bf16, isOutput=False)
    wkT = nc.declare_dram_parameter("wkT", [D, EP], bf16, isOutput=False)
    wvT = nc.declare_dram_parameter("wvT", [D, DV], bf16, isOutput=False)
    woT = nc.declare_dram_parameter("woT", [DV, D], f32r, isOutput=False)
    cosN = nc.declare_dram_parameter("cosN", [T, HD], bf16, isOutput=False)
    identR = nc.declare_dram_parameter("identR", [128, 128], f32r, isOutput=False)
    identB = nc.declare_dram_parameter("identB", [128, 128], bf16, isOutput=False)
    sinN = nc.declare_dram_parameter("sinN", [T, HD], bf16, isOutput=False)
    out = nc.declare_dram_parameter("out", [T, D], f32, isOutput=True)

    def rope(qraw, qtl, cos3, sin3, tmps):
        """qraw [128, EP] f32 -> qtl [128, EP] bf16 with rotary applied.

        Column map: head h dim e<128 -> col 128h+e ; dim 128+j -> col 512+32h+j.
        rot_half partner: e<72 -> e+72 (sign -), e>=72 -> e-72 (sign +).
        cos3/sin3: [128, 4(bcast), 144] broadcast views of this t-tile's rows.
        Two full products m1=q*cos, m2=q*sin (4 ops), then 4 region combines.
        """
        qa = qraw[:, 0:512].rearrange("p (h e) -> p h e", h=HL)
        qb = qraw[:, 512:EP].rearrange("p (h e) -> p h e", h=HL)
        oa = qtl[:, 0:512].rearrange("p (h e) -> p h e", h=HL)
        ob = qtl[:, 512:EP].rearrange("p (h e) -> p h e", h=HL)
        m1, m2 = tmps
        m1a = m1[:, 0:512].rearrange("p (h e) -> p h e", h=HL)
        m1b = m1[:, 512:EP].rearrange("p (h e) -> p h e", h=HL)
        m2a = m2[:, 0:512].rearrange("p (h e) -> p h e", h=HL)
        m2b = m2[:, 512:EP].rearrange("p (h e) -> p h e", h=HL)
        v = nc.vector
        v.tensor_mul(m1a[:, :, 0:128], qa[:, :, 0:128], cos3[:, :, 0:128])
        v.tensor_mul(m1b[:, :, 0:16], qb[:, :, 0:16], cos3[:, :, 128:144])
        # m2[j] = q[j] * sin[partner(j)] so combines read m2 at the partner col
        v.tensor_mul(m2a[:, :, 0:56], qa[:, :, 0:56], sin3[:, :, 72:128])
        v.tensor_mul(m2a[:, :, 56:72], qa[:, :, 56:72], sin3[:, :, 128:144])
        v.tensor_mul(m2a[:, :, 72:128], qa[:, :, 72:128], sin3[:, :, 0:56])
        v.tensor_mul(m2b[:, :, 0:16], qb[:, :, 0:16], sin3[:, :, 56:72])
        # e in [0,56):  out = m1[e] - m2[e+72]
        v.tensor_sub(oa[:, :, 0:56], m1a[:, :, 0:56], m2a[:, :, 72:128])
        # e in [56,72): partner lives in the b block
        v.tensor_sub(oa[:, :, 56:72], m1a[:, :, 56:72], m2b[:, :, 0:16])
        # e in [72,128): out = m1[e] + m2[e-72]
        v.tensor_add(oa[:, :, 72:128], m1a[:, :, 72:128], m2a[:, :, 0:56])
        # e in [128,144): out = m1b + m2[56:72]
        v.tensor_add(ob[:, :, 0:16], m1b[:, :, 0:16], m2a[:, :, 56:72])
        # zero the 16 pad cols of each head's b-block
        v.memset(ob[:, :, 16:32], 0.0)

    with TileContext(nc) as tc:
        with tc.tile_pool(name="persist", bufs=1) as P0:
            ident_bf = P0.tile([128, 128], bf16, name="ident_bf", tag="ident_bf")
            ident_f32 = P0.tile([128, 128], f32r, name="ident_f32", tag="ident_f32")
            nc.sync.dma_start(ident_bf[:], identB[:])
            nc.sync.dma_start(ident_f32[:], identR[:])

            qTa = [
                P0.tile([128, T], bf16, name=f"qTa{h}", tag=f"qTa{h}")
                for h in range(HL)
            ]
            kTa = [
                P0.tile([128, T], bf16, name=f"kTa{h}", tag=f"kTa{h}")
                for h in range(HL)
            ]
            qTB = P0.tile([128, T], bf16, name="qTB", tag="qTB")
            kTB = P0.tile([128, T], bf16, name="kTB", tag="kTB")
            # per-head replicas of the b-block rows at all four 32-row groups,
            # so four consecutive key-tiles' K=32 score matmuls can issue to
            # distinct PE row-groups and overlap in the array
            qTBr = [
                P0.tile([128, T], bf16, name=f"qTBr{h}", tag=f"qTBr{h}")
                for h in range(HL)
            ]
            kTBr = [
                P0.tile([128, T], bf16, name=f"kTBr{h}", tag=f"kTBr{h}")
                for h in range(HL)
            ]
            vt = [
                P0.tile([128, HL * (HD + 1)], bf16, name=f"v{t}", tag=f"v{t}")
                for t in range(NT)
            ]

            # ---------------- Phase A: projections + rope + transposes -----
            with (
                tc.tile_pool(name="pa", bufs=1) as pa,
                tc.tile_pool(name="paps", bufs=1, space="PSUM") as paps,
            ):
                xtiles = [
                    pa.tile([128, T], bf16, name=f"xTs{k}", tag=f"xTs{k}")
                    for k in range(KC)
                ]
                cos_sb = pa.tile([128, NT * HD], bf16, name="cos_sb", tag="cos_sb")
                sin_sb = pa.tile([128, NT * HD], bf16, name="sin_sb", tag="sin_sb")

                def trig3(sb, n):
                    # [128, 144] row block for t-tile n, broadcast over 4 heads
                    return (
                        sb[:, n * HD : (n + 1) * HD]
                        .rearrange("p (o r) -> p o r", o=1)
                        .to_broadcast([128, HL, HD])
                    )

                def proj_phase(wdram, width, consume_head, consume_tail, first=False):
                    wtiles = []
                    for k in range(KC):
                        wt_ = pa.tile(
                            [128, EP], bf16, name=f"w{k}", tag=f"W{k}"
                        )
                        nsp = 2 if (first and k < 3) else 1
                        w_ = width // nsp
                        for j in range(nsp):
                            nc.sync.dma_start(
                                wt_[:, j * w_ : (j + 1) * w_],
                                wdram[k * 128 : (k + 1) * 128, j * w_ : (j + 1) * w_],
                            )
                        wtiles.append(wt_)
                        if first:
                            # interleave the x chunk right after its weight
                            # chunk so matmul k can start as soon as pair k
                            # lands, instead of waiting for the whole load
                            nsp = 8 if k == 0 else (4 if k < 3 else 2)
                            w_ = T // nsp
                            for j in range(nsp):
                                nc.sync.dma_start(
                                    xtiles[k][:, j * w_ : (j + 1) * w_],
                                    xT[
                                        k * 128 : (k + 1) * 128,
                                        j * w_ : (j + 1) * w_,
                                    ],
                                )
                    half = width // 2
                    pending = None
                    for n in range(NT):
                        ps0 = paps.tile([128, 320], f32, name="ps0", tag="proj", bufs=6)
                        ps1 = paps.tile([128, 320], f32, name="ps1", tag="proj", bufs=6)
                        for k in range(KC):
                            st, sp = k == 0, k == KC - 1
                            lhs = xtiles[k][:, n * 128 : (n + 1) * 128]
                            nc.tensor.matmul(
                                ps0[:, 0:half],
                                lhs,
                                wtiles[k][:, 0:half],
                                start=st,
                                stop=sp,
                            )
                            nc.tensor.matmul(
                                ps1[:, 0:half],
                                lhs,
                                wtiles[k][:, half:width],
                                start=st,
                                stop=sp,
                            )
                        if pending is not None:
                            consume_tail(*pending)
                            pending = None
                        carry = consume_head(n, ps0[:, 0:half], ps1[:, 0:half])
                        if consume_tail is not None:
                            pending = (n, carry)
                    if pending is not None:
                        consume_tail(*pending)

                def qk_consume(qtl_dst_a, qtl_dst_b):
                    def head(n, ps0, ps1):
                        qraw = pa.tile([128, EP], f32, name="qraw", tag="qraw", bufs=3)
                        nc.any.tensor_copy(qraw[:, 0:320], ps0)
                        nc.any.tensor_copy(qraw[:, 320:EP], ps1)
                        qtl = pa.tile([128, EP], bf16, name="qtl", tag="qtl", bufs=3)
                        tA = pa.tile([128, EP], f32, name="ropeA", tag="ropeA", bufs=2)
                        tB = pa.tile([128, EP], f32, name="ropeB", tag="ropeB", bufs=2)
                        rope(qraw, qtl, trig3(cos_sb, n), trig3(sin_sb, n), (tA, tB))
                        return qtl

                    def tail(n, qtl):
                        for j in range(5):
                            tp = paps.tile(
                                [128, 128], bf16, name="tp", tag="tp", bufs=2
                            )
                            nc.tensor.transpose(
                                tp[:], qtl[:, 128 * j : 128 * (j + 1)], ident_bf[:]
                            )
                            dst = qtl_dst_a[j] if j < 4 else qtl_dst_b
                            nc.any.tensor_copy(
                                dst[:, n * 128 : (n + 1) * 128], tp[:]
                            )

                    return head, tail

                def v_consume(n, ps0, ps1):
                    v3 = vt[n].rearrange("p (h e) -> p h e", h=HL)
                    nc.any.tensor_copy(
                        v3[:, 0:2, 0:HD],
                        ps0.rearrange("p (h e) -> p h e", h=2),
                    )
                    nc.any.tensor_copy(
                        v3[:, 2:4, 0:HD],
                        ps1.rearrange("p (h e) -> p h e", h=2),
                    )
                    nc.vector.memset(v3[:, :, HD : HD + 1], 1.0)

                qh, qt_ = qk_consume(qTa, qTB)
                kh, kt_ = qk_consume(kTa, kTB)
                proj_phase(wvT, DV, v_consume, None, first=True)
                nc.sync.dma_start(
                    cos_sb.rearrange("p (n r) -> p n r", n=NT),
                    cosN.rearrange("(n p) r -> p n r", p=128),
                )
                nc.sync.dma_start(
                    sin_sb.rearrange("p (n r) -> p n r", n=NT),
                    sinN.rearrange("(n p) r -> p n r", p=128),
                )
                proj_phase(wqT, EP, qh, qt_)
                proj_phase(wkT, EP, kh, kt_)
                # replicate the b-blocks after both phases so these DMAs don't
                # delay the k-phase weight loads; h-major so head 0 lands first
                for hh in range(HL):
                    for j in range(4):
                        nc.sync.dma_start(
                            qTBr[hh][32 * j : 32 * j + 32, :],
                            qTB[32 * hh : 32 * hh + 32, :],
                        )
                        nc.sync.dma_start(
                            kTBr[hh][32 * j : 32 * j + 32, :],
                            kTB[32 * hh : 32 * hh + 32, :],
                        )

            # ---------------- Phase B: attention --------------------------
            with tc.tile_pool(name="pb", bufs=1) as pb:
                ot = [
                    pb.tile([128, DV], f32r, name=f"o{t}", tag=f"o{t}")
                    for t in range(NT)
                ]
                with tc.tile_pool(name="pbps", bufs=1, space="PSUM") as pbps:
                    for qb in range(4):
                        for h in range(HL):
                            # pack the 4 q-tile accumulators into 2 PSUM banks:
                            # 3*145 fp32 = 1740B fits one 2KB bank
                            o_ps3 = pbps.tile(
                                [128, 3 * (HD + 1)], f32, name="o_ps3", tag="o3", bufs=2
                            )
                            o_ps1 = pbps.tile(
                                [128, HD + 1], f32, name="o_ps1", tag="o1", bufs=2
                            )
                            o_ps = [
                                o_ps3[:, 0 : HD + 1],
                                o_ps3[:, HD + 1 : 2 * (HD + 1)],
                                o_ps3[:, 2 * (HD + 1) : 3 * (HD + 1)],
                                o_ps1[:],
                            ]

                            def s_exp_group(g):
                                # 4 key-tiles per group: the K=128 head MMs,
                                # then the four K=32 tail MMs back-to-back on
                                # row-groups 0/32/64/96 (concurrent in PE)
                                spss = [
                                    pbps.tile(
                                        [128, 512], f32, name="sps", tag="sc", bufs=4
                                    )
                                    for _ in range(gsz)
                                ]
                                for j in range(gsz):
                                    kt = gsz * g + j
                                    nc.tensor.matmul(
                                        spss[j][:],
                                        kTa[h][:, kt * 128 : (kt + 1) * 128],
                                        qTa[h][:, qb * 512 : (qb + 1) * 512],
                                        start=True,
                                        stop=False,
                                    )
                                for j in range(gsz):
                                    kt = gsz * g + j
                                    nc.tensor.matmul(
                                        spss[j][:],
                                        kTBr[h][32 * j : 32 * j + 32, kt * 128 : (kt + 1) * 128],
                                        qTBr[h][32 * j : 32 * j + 32, qb * 512 : (qb + 1) * 512],
                                        start=False,
                                        stop=True,
                                        tile_position=(32 * j, 0),
                                    )
                                Es = []
                                for j in range(gsz):
                                    E = pb.tile(
                                        [128, 512], bf16, name="E", tag="E", bufs=max(4, 2 * gsz)
                                    )
                                    nc.scalar.activation(
                                        E[:], spss[j][:], AF.Exp, scale=SCALE
                                    )
                                    Es.append(E)
                                return Es

                            def pv_group(g, Es):
                                for j in range(gsz):
                                    kt = gsz * g + j
                                    for qt in range(4):
                                        # start/stop are bank-granular: qt 0-2
                                        # share o_ps3's bank, so only the
                                        # first/last bank write carries them
                                        if qt < 3:
                                            st = kt == 0 and qt == 0
                                            sp = kt == NT - 1 and qt == 2
                                        else:
                                            st = kt == 0
                                            sp = kt == NT - 1
                                        nc.tensor.matmul(
                                            o_ps[qt][:],
                                            Es[j][:, qt * 128 : (qt + 1) * 128],
                                            vt[kt][:, (HD + 1) * h : (HD + 1) * (h + 1)],
                                            start=st,
                                            stop=sp,
                                        )

                            ngrp = NT // gsz
                            Eprev = s_exp_group(0)
                            for g in range(ngrp):
                                Enext = s_exp_group(g + 1) if g + 1 < ngrp else None
                                pv_group(g, Eprev)
                                Eprev = Enext
                            for qt in range(4):
                                t = qb * 4 + qt
                                r = pb.tile([128, 1], f32, name="r", tag="r", bufs=4)
                                nc.vector.reciprocal(r[:], o_ps[qt][:, HD : HD + 1])
                                nc.vector.tensor_scalar_mul(
                                    ot[t][:, HD * h : HD * (h + 1)],
                                    o_ps[qt][:, 0:HD],
                                    r[:],
                                )

                # ---------------- Phase C: o^T + final projection ----------
                oTa = [
                    pb.tile([128, T], f32r, name=f"oTa{j}", tag=f"oTa{j}")
                    for j in range(4)
                ]
                oTb = pb.tile([64, T], f32r, name="oTb", tag="oTb")
                wo_tiles = []
                for k in range(5):
                    rows = 128 if k < 4 else 64
                    wot_ = pb.tile([128, D], f32r, name=f"wo{k}", tag=f"wo{k}")
                    nc.sync.dma_start(
                        wot_[0:rows, :], woT[k * 128 : k * 128 + rows, :]
                    )
                    wo_tiles.append(wot_)
                with tc.tile_pool(name="pcps", bufs=1, space="PSUM") as pcps:

                    def o_transp(t):
                        for j in range(4):
                            tp = pcps.tile(
                                [128, 128], f32r, name="tpo", tag="otp", bufs=3
                            )
                            nc.tensor.transpose(
                                tp[:],
                                ot[t][:, 128 * j : 128 * (j + 1)],
                                ident_f32[:],
                            )
                            nc.any.tensor_copy(
                                oTa[j][:, t * 128 : (t + 1) * 128], tp[:]
                            )
                        tpb = pcps.tile([64, 128], f32r, name="tpb", tag="otp", bufs=3)
                        nc.tensor.transpose(
                            tpb[:],
                            ot[t][:, 512:DV],
                            ident_f32[:],
                        )
                        nc.any.tensor_copy(
                            oTb[:, t * 128 : (t + 1) * 128], tpb[:]
                        )

                    def final(t):
                        for j3 in range(3):
                            fps = pcps.tile([128, 384], f32, name="fps", tag="f", bufs=3)
                            for k in range(5):
                                lhs = (
                                    oTa[k][:, t * 128 : (t + 1) * 128]
                                    if k < 4
                                    else oTb[:, t * 128 : (t + 1) * 128]
                                )
                                nc.tensor.matmul(
                                    fps[:],
                                    lhs,
                                    wo_tiles[k][
                                        0 : (128 if k < 4 else 64),
                                        384 * j3 : 384 * (j3 + 1),
                                    ],
                                    start=(k == 0),
                                    stop=(k == 4),
                                )
                            fout = pb.tile(
                                [128, 384], f32, name="fout", tag="fout", bufs=4
                            )
                            nc.any.tensor_copy(fout[:], fps[:])
                            nc.sync.dma_start(
                                out[
                                    t * 128 : (t + 1) * 128,
                                    384 * j3 : 384 * (j3 + 1),
                                ],
                                fout[:],
                            )

                    o_transp(0)
                    for t in range(NT):
                        if t + 1 < NT:
                            o_transp(t + 1)
                        final(t)

    nc.compile()
    return nc


def get_nc(debug=False, gsz=None):
    key = (bool(debug), GSZ if gsz is None else gsz)
    if key not in _NC_CACHE:
        _NC_CACHE[key] = _build(debug, gsz)
    return _NC_CACHE[key]


def make_in_maps(x, cos, sin, Wq, Wk, Wv, Wo):
    import ml_dtypes

    x = np.asarray(x, np.float32)
    cos = np.asarray(cos, np.float32)
    sin = np.asarray(sin, np.float32)
    Wq, Wk, Wv, Wo = (np.asarray(w, np.float32) for w in (Wq, Wk, Wv, Wo))
    cos_bf = cos.astype(ml_dtypes.bfloat16)
    sin_bf = sin.astype(ml_dtypes.bfloat16)

    in_maps = []
    for c in range(NCORES):
        b, hg = divmod(c, 2)
        heads = [HL * hg + i for i in range(HL)]

        def qk_w(W):
            Wsel = np.zeros((EP, D), np.float32)
            for i, g in enumerate(heads):
                Wsel[128 * i : 128 * i + 128] = W[144 * g : 144 * g + 128]
                Wsel[512 + 32 * i : 512 + 32 * i + 16] = W[144 * g + 128 : 144 * g + 144]
            return np.ascontiguousarray(Wsel.T)

        wv_sel = np.concatenate([Wv[144 * g : 144 * g + 144] for g in heads], 0)
        wo_sel = np.concatenate([Wo[:, 144 * g : 144 * g + 144] for g in heads], 1)
        in_maps.append(
            {
                "xT": np.ascontiguousarray(x[b].T).astype(ml_dtypes.bfloat16),
                "wqT": qk_w(Wq).astype(ml_dtypes.bfloat16),
                "wkT": qk_w(Wk).astype(ml_dtypes.bfloat16),
                "wvT": np.ascontiguousarray(wv_sel.T).astype(ml_dtypes.bfloat16),
                "woT": np.ascontiguousarray(wo_sel.T),
                "cosN": cos_bf,
                "sinN": sin_bf,
                "identR": np.eye(128, dtype=np.float32),
                "identB": np.eye(128, dtype=ml_dtypes.bfloat16),
            }
        )
    return in_maps


def kernel(x, cos, sin, Wq, Wk, Wv, Wo, _trace=False, _trace_kwargs=None):
    from concourse.bass_utils import run_bass_kernel_spmd

    nc = get_nc()
    in_maps = make_in_maps(x, cos, sin, Wq, Wk, Wv, Wo)
    res = run_bass_kernel_spmd(
        nc,
        in_maps,
        list(range(NCORES)),
        trace=_trace,
        **(_trace_kwargs or {}),
    )
    parts = [res.results[c]["out"] for c in range(NCORES)]
    outb = np.stack([parts[2 * b] + parts[2 * b + 1] for b in range(B)])
    if _trace:
        kernel.last_results = res
    return outb.astype(np.float32)



# revision 6
# speedup vs baseline: 1.2410x; 1.2410x over previous
"""Trainium2 Bass kernel for a fused multi-head attention block.

Reference computation (B=4, T=2048, D=1152, H=8, HD=144, full rotary):
    q,k,v = x@Wq.T, x@Wk.T, x@Wv.T   (per head)
    q,k   = rope(q, k, cos, sin)
    o     = softmax(q k^T / sqrt(HD)) v
    out   = o @ Wo.T
Sharding (8 cores): core c = (batch b = c//2, head-group hg = c%2).
Each core computes 4 heads of one batch and a partial output
out_part = o_local @ Wo[:, hg_cols].T ; host sums the two partials per batch.

v2 design (vs. the transpose-heavy v1):
  * q/k are projected DIRECTLY transposed: qT = Wsel^T-chunks (stationary)
    x xT (moving), so the scores layout [dim, T] needs no PE transposes.
  * rope runs in the transposed layout: the rotate-half partner lives at a
    partition offset, so a DMA SBUF->SBUF copy builds a partition-shifted
    replica qS with qS[e] = q[partner(e)]; then
    q_rot = q * cosT + qS * sinT_signed  (3 bf16 tensor_tensor ops, the
    rotate-half signs are folded into sinT_signed on the host).
  * per-head packing (same as v1): heads' dims 0..127 in four [128,T] tiles,
    dims 128..143 in a shared b-block tile at rows 32h..32h+16 (+16 zero pad),
    replicated to all four 32-row groups so the K=32 score-tail matmuls can
    run concurrently via tile_position.
  * scores S^T accumulate in a [128,1024] PSUM tile (two banks, two
    key-tiles per group) so each Exp activation covers 1024 elems/partition —
    halving ScalarE instruction overhead vs. [128,512] activations.
  * softmax denominator via ones-column appended to v (o_ps[:,144]).
  * phase C (oT transpose + final projection) runs fully in bf16 and all
    PSUM->SBUF evacuations go to Vector/Scalar (never the same engine that
    gates the PE), keeping the PE stream dense so HAM stays at K=8/8.
  * 16 zero matmuls at t=0 prime the PE activity monitor while the first
    DMAs land; a dummy Exp preloads the ACT table set.
"""

import numpy as np

B, T, D, H = 4, 2048, 1152, 8
HL = 4              # heads per core
HD = 144            # head dim
EP = 640            # packed q/k width: 4*128 + 128 (4x(16+16pad))
DV = HL * HD        # 576, v/o width
VW = HL * (HD + 1)  # 580, v + ones col
NT = T // 128       # 16 t-tiles
KC = D // 128       # 9 contraction chunks
SCALE = float(HD) ** -0.5
NCORES = 8

_NC_CACHE = {}


def _build(debug=False):
    import concourse.bacc as bacc
    import concourse.mybir as mybir
    from concourse.tile import TileContext

    dt = mybir.dt
    f32, bf16 = dt.float32, dt.bfloat16
    AF = mybir.ActivationFunctionType

    nc = bacc.Bacc(
        "TRN2",
        target_bir_lowering=False,
        debug=debug,
        enable_asserts=False,
        num_devices=NCORES,
    )

    xT = nc.declare_dram_parameter("xT", [D, T], bf16, isOutput=False)
    wqT = nc.declare_dram_parameter("wqT", [D, EP], bf16, isOutput=False)
    wkT = nc.declare_dram_parameter("wkT", [D, EP], bf16, isOutput=False)
    wvT = nc.declare_dram_parameter("wvT", [D, DV], bf16, isOutput=False)
    woT = nc.declare_dram_parameter("woT", [DV, D], bf16, isOutput=False)
    cosTa = nc.declare_dram_parameter("cosTa", [128, T], bf16, isOutput=False)
    sinTa = nc.declare_dram_parameter("sinTa", [128, T], bf16, isOutput=False)
    cosTb = nc.declare_dram_parameter("cosTb", [128, T], bf16, isOutput=False)
    sinTb = nc.declare_dram_parameter("sinTb", [128, T], bf16, isOutput=False)
    identB = nc.declare_dram_parameter("identB", [128, 128], bf16, isOutput=False)
    out = nc.declare_dram_parameter("out", [T, D], f32, isOutput=True)

    with TileContext(nc) as tc:
        with tc.tile_pool(name="persist", bufs=1) as P0:
            ident_bf = P0.tile([128, 128], bf16, name="ident_bf", tag="ident_bf")
            nc.sync.dma_start(ident_bf[:], identB[:])

            qTa = [P0.tile([128, T], bf16, name=f"qTa{h}", tag=f"qTa{h}")
                   for h in range(HL)]
            kTa = [P0.tile([128, T], bf16, name=f"kTa{h}", tag=f"kTa{h}")
                   for h in range(HL)]
            qTBr = [P0.tile([128, T], bf16, name=f"qTBr{h}", tag=f"qTBr{h}")
                    for h in range(HL)]
            kTBr = [P0.tile([128, T], bf16, name=f"kTBr{h}", tag=f"kTBr{h}")
                    for h in range(HL)]
            vt = [P0.tile([128, VW], bf16, name=f"v{t}", tag=f"v{t}")
                  for t in range(NT)]

            # ------------- Phase A: transposed projections + rope ----------
            with (
                tc.tile_pool(name="pa", bufs=1) as pa,
                tc.tile_pool(name="paps", bufs=1, space="PSUM") as paps,
            ):
                # PE warm-up: zero matmuls keep the activity monitor busy
                # while the first weight/x DMAs land.
                wup = pa.tile([128, 512], bf16, name="wup", tag="wup")
                nc.vector.memset(wup[:], 0.0)
                for _ in range(16):
                    wps = paps.tile([128, 512], f32, name="wps", tag="wps", bufs=2)
                    nc.tensor.matmul(wps[:], wup[:, 0:128], wup[:],
                                     start=True, stop=True)
                # preload the exp table set early (one-time ~2.7us)
                dumm = pa.tile([128, 8], f32, name="dumm", tag="dumm")
                nc.scalar.activation(dumm[:], wup[:, 0:8], AF.Exp)

                xt = [pa.tile([128, T], bf16, name=f"xt{k}", tag=f"xt{k}")
                      for k in range(KC)]
                cos_a = pa.tile([128, T], bf16, name="cos_a", tag="cos_a")
                sin_a = pa.tile([128, T], bf16, name="sin_a", tag="sin_a")
                cos_b = pa.tile([128, T], bf16, name="cos_b", tag="cos_b")
                sin_b = pa.tile([128, T], bf16, name="sin_b", tag="sin_b")

                def qk_phase(wdram, dstA, dstBr, first=False):
                    wsb = []
                    for k in range(KC):
                        wt = pa.tile([128, EP], bf16, name=f"w{k}", tag=f"W{k}")
                        # b-cols first: the B block is projected first
                        nc.sync.dma_start(
                            wt[:, 512:EP], wdram[k * 128:(k + 1) * 128, 512:EP])
                        nc.sync.dma_start(
                            wt[:, 0:512], wdram[k * 128:(k + 1) * 128, 0:512])
                        wsb.append(wt)
                        if first:
                            nsp = 4 if k < 2 else 2
                            w_ = T // nsp
                            for j in range(nsp):
                                nc.sync.dma_start(
                                    xt[k][:, j * w_:(j + 1) * w_],
                                    xT[k * 128:(k + 1) * 128, j * w_:(j + 1) * w_])

                    def proj_block(cols, dst):
                        # dst[:, :] (bf16 SBUF) <- (wsb[:, cols]).T @ xt
                        for tg in range(4):
                            ps = paps.tile([128, 512], f32, name="pps",
                                           tag="projps", bufs=4)
                            for k in range(KC):
                                nc.tensor.matmul(
                                    ps[:], wsb[k][:, cols],
                                    xt[k][:, tg * 512:(tg + 1) * 512],
                                    start=(k == 0), stop=(k == KC - 1))
                            nc.scalar.copy(dst[:, tg * 512:(tg + 1) * 512], ps[:])

                    rawB = pa.tile([128, T], bf16, name="rawB", tag="rawB")
                    proj_block(slice(512, EP), rawB)
                    qSB = pa.tile([128, T], bf16, name="qSB", tag="qSB")
                    nc.gpsimd.memset(qSB[:], 0.0)

                    for h in range(HL):
                        rawA = pa.tile([128, T], bf16, name="rawA",
                                       tag="rawA", bufs=2)
                        proj_block(slice(h * 128, (h + 1) * 128), rawA)
                        # partition-shifted replica qS[e] = raw[partner(e)]
                        qS = pa.tile([128, T], bf16, name="qS", tag="qS", bufs=2)
                        nc.gpsimd.dma_start(qS[0:56, :], rawA[72:128, :])
                        nc.gpsimd.dma_start(qS[56:72, :], rawB[32 * h:32 * h + 16, :])
                        nc.gpsimd.dma_start(qS[72:128, :], rawA[0:56, :])
                        nc.gpsimd.dma_start(qSB[32 * h:32 * h + 16, :], rawA[56:72, :])
                        eng = nc.vector if h % 2 == 0 else nc.gpsimd
                        m1 = pa.tile([128, T], bf16, name="m1", tag="m1", bufs=2)
                        m2 = pa.tile([128, T], bf16, name="m2", tag="m2", bufs=2)
                        eng.tensor_mul(m1[:], qS[:], sin_a[:])
                        eng.tensor_mul(m2[:], rawA[:], cos_a[:])
                        eng.tensor_add(dstA[h][:], m1[:], m2[:])

                    # b-block rope + 4x row-group replication
                    mB1 = pa.tile([128, T], bf16, name="mB1", tag="m1", bufs=2)
                    mB2 = pa.tile([128, T], bf16, name="mB2", tag="m2", bufs=2)
                    qTB = pa.tile([128, T], bf16, name="qTB", tag="qTB")
                    nc.vector.tensor_mul(mB1[:], qSB[:], sin_b[:])
                    nc.vector.tensor_mul(mB2[:], rawB[:], cos_b[:])
                    nc.vector.tensor_add(qTB[:], mB1[:], mB2[:])
                    for h in range(HL):
                        for j in range(4):
                            nc.gpsimd.dma_start(
                                dstBr[h][32 * j:32 * j + 32, :],
                                qTB[32 * h:32 * h + 32, :])

                nc.scalar.dma_start(cos_a[:], cosTa[:])
                nc.scalar.dma_start(sin_a[:], sinTa[:])
                nc.scalar.dma_start(cos_b[:], cosTb[:])
                nc.scalar.dma_start(sin_b[:], sinTb[:])
                # q first, then k, then v: each phase's b-block rope +
                # replication tail is hidden under the next phase's matmuls,
                # so the first attention group starts with all deps ready.
                qk_phase(wqT, qTa, qTBr, first=True)
                qk_phase(wkT, kTa, kTBr)

                # ---- v projection (natural [t, e] layout) ----
                wv_sb = []
                for k in range(KC):
                    wt = pa.tile([128, DV], bf16, name=f"wv{k}", tag=f"W{k}")
                    nc.sync.dma_start(wt[:], wvT[k * 128:(k + 1) * 128, :])
                    wv_sb.append(wt)
                for n in range(NT):
                    ps0 = paps.tile([128, 288], f32, name="ps0", tag="projps", bufs=4)
                    ps1 = paps.tile([128, 288], f32, name="ps1", tag="projps", bufs=4)
                    for k in range(KC):
                        lhs = xt[k][:, n * 128:(n + 1) * 128]
                        nc.tensor.matmul(ps0[:], lhs, wv_sb[k][:, 0:288],
                                         start=(k == 0), stop=(k == KC - 1))
                        nc.tensor.matmul(ps1[:], lhs, wv_sb[k][:, 288:DV],
                                         start=(k == 0), stop=(k == KC - 1))
                    v3 = vt[n].rearrange("p (h e) -> p h e", h=HL)
                    nc.scalar.copy(v3[:, 0:2, 0:HD],
                                   ps0.rearrange("p (h e) -> p h e", h=2))
                    nc.scalar.copy(v3[:, 2:4, 0:HD],
                                   ps1.rearrange("p (h e) -> p h e", h=2))
                    nc.vector.memset(v3[:, :, HD:HD + 1], 1.0)

            # ------------- Phase B: attention ------------------------------
            with tc.tile_pool(name="pb", bufs=1) as pb:
                ot = [pb.tile([128, DV], bf16, name=f"o{t}", tag=f"o{t}")
                      for t in range(NT)]
                with tc.tile_pool(name="pbps", bufs=1, space="PSUM") as pbps:
                    for qb in range(4):
                        for h in range(HL):
                            o_ps3 = pbps.tile([128, 3 * (HD + 1)], f32,
                                              name="o_ps3", tag="o3", bufs=1)
                            o_ps1 = pbps.tile([128, HD + 1], f32,
                                              name="o_ps1", tag="o1", bufs=1)
                            o_ps = [
                                o_ps3[:, 0:HD + 1],
                                o_ps3[:, HD + 1:2 * (HD + 1)],
                                o_ps3[:, 2 * (HD + 1):3 * (HD + 1)],
                                o_ps1[:],
                            ]

                            def s_exp(g):
                                sps = pbps.tile([128, 1024], f32, name="sps",
                                                tag="sc", bufs=3)
                                for j in range(2):
                                    kt = 2 * g + j
                                    nc.tensor.matmul(
                                        sps[:, j * 512:(j + 1) * 512],
                                        kTa[h][:, kt * 128:(kt + 1) * 128],
                                        qTa[h][:, qb * 512:(qb + 1) * 512],
                                        start=True, stop=False)
                                for j in range(2):
                                    kt = 2 * g + j
                                    rg = kt % 4
                                    nc.tensor.matmul(
                                        sps[:, j * 512:(j + 1) * 512],
                                        kTBr[h][32 * rg:32 * rg + 32,
                                                kt * 128:(kt + 1) * 128],
                                        qTBr[h][32 * rg:32 * rg + 32,
                                                qb * 512:(qb + 1) * 512],
                                        start=False, stop=True,
                                        tile_position=(32 * rg, 0))
                                E = pb.tile([128, 1024], bf16, name="E",
                                            tag="E", bufs=4)
                                nc.scalar.activation(E[:], sps[:], AF.Exp,
                                                     scale=SCALE)
                                return E

                            def pv(g, E):
                                for j in range(2):
                                    kt = 2 * g + j
                                    for qt in range(4):
                                        if qt < 3:
                                            st = kt == 0 and qt == 0
                                            sp = kt == NT - 1 and qt == 2
                                        else:
                                            st = kt == 0
                                            sp = kt == NT - 1
                                        nc.tensor.matmul(
                                            o_ps[qt][:],
                                            E[:, j * 512 + qt * 128:
                                              j * 512 + (qt + 1) * 128],
                                            vt[kt][:, (HD + 1) * h:
                                                   (HD + 1) * (h + 1)],
                                            start=st, stop=sp)

                            ngrp = NT // 2
                            Ep = s_exp(0)
                            for g in range(ngrp):
                                En = s_exp(g + 1) if g + 1 < ngrp else None
                                pv(g, Ep)
                                Ep = En
                            for qt in range(4):
                                t = qb * 4 + qt
                                r = pb.tile([128, 1], f32, name="r", tag="r",
                                            bufs=4)
                                nc.vector.reciprocal(r[:], o_ps[qt][:, HD:HD + 1])
                                nc.vector.tensor_scalar_mul(
                                    ot[t][:, HD * h:HD * (h + 1)],
                                    o_ps[qt][:, 0:HD], r[:])

                # ------------- Phase C: o^T + final projection -------------
                oTa = [pb.tile([128, T], bf16, name=f"oTa{j}", tag=f"oTa{j}")
                       for j in range(4)]
                oTb = pb.tile([64, T], bf16, name="oTb", tag="oTb")
                wo_sb = []
                for k in range(5):
                    rows = 128 if k < 4 else 64
                    wot = pb.tile([128, D], bf16, name=f"wo{k}", tag=f"wo{k}")
                    nc.sync.dma_start(wot[0:rows, :], woT[k * 128:k * 128 + rows, :])
                    wo_sb.append(wot)
                with tc.tile_pool(name="pcps", bufs=1, space="PSUM") as pcps:

                    def o_transp(t):
                        for j in range(4):
                            tp = pcps.tile([128, 128], bf16, name="tpo",
                                           tag="otp", bufs=4)
                            nc.tensor.transpose(
                                tp[:], ot[t][:, 128 * j:128 * (j + 1)],
                                ident_bf[:])
                            nc.scalar.copy(oTa[j][:, t * 128:(t + 1) * 128], tp[:])
                        tpb = pcps.tile([64, 128], bf16, name="tpb",
                                        tag="otp", bufs=4)
                        nc.tensor.transpose(tpb[:], ot[t][:, 512:DV], ident_bf[:])
                        nc.scalar.copy(oTb[:, t * 128:(t + 1) * 128], tpb[:])

                    def final(t):
                        for j3 in range(3):
                            fps = pcps.tile([128, 384], f32, name="fps",
                                            tag="f", bufs=3)
                            for k in range(5):
                                lhs = (oTa[k][:, t * 128:(t + 1) * 128]
                                       if k < 4
                                       else oTb[:, t * 128:(t + 1) * 128])
                                nc.tensor.matmul(
                                    fps[:], lhs,
                                    wo_sb[k][0:(128 if k < 4 else 64),
                                             384 * j3:384 * (j3 + 1)],
                                    start=(k == 0), stop=(k == 4))
                            fout = pb.tile([128, 384], f32, name="fout",
                                           tag="fout", bufs=4)
                            nc.vector.tensor_copy(fout[:], fps[:])
                            nc.sync.dma_start(
                                out[t * 128:(t + 1) * 128,
                                    384 * j3:384 * (j3 + 1)], fout[:])

                    o_transp(0)
                    for t in range(NT):
                        if t + 1 < NT:
                            o_transp(t + 1)
                        final(t)

    nc.compile()
    return nc


def get_nc(debug=False):
    key = bool(debug)
    if key not in _NC_CACHE:
        _NC_CACHE[key] = _build(debug)
    return _NC_CACHE[key]


def make_in_maps(x, cos, sin, Wq, Wk, Wv, Wo):
    import ml_dtypes

    x = np.asarray(x, np.float32)
    cos = np.asarray(cos, np.float32)
    sin = np.asarray(sin, np.float32)
    Wq, Wk, Wv, Wo = (np.asarray(w, np.float32) for w in (Wq, Wk, Wv, Wo))

    # transposed trig tables with the rotate-half signs folded in:
    # out[e] = raw[e]*cos[e] + sgn(e)*raw[partner(e)]*sin[e]
    cosT = np.ascontiguousarray(cos.T)   # [144, T]
    sinT = np.ascontiguousarray(sin.T)
    sgn = np.ones((128, 1), np.float32)
    sgn[:72] = -1.0
    cosTa = cosT[0:128]
    sinTa = sinT[0:128] * sgn
    cosTb = np.zeros((128, T), np.float32)
    sinTb = np.zeros((128, T), np.float32)
    for hh in range(HL):
        cosTb[32 * hh:32 * hh + 16] = cosT[128:144]
        sinTb[32 * hh:32 * hh + 16] = sinT[128:144]
    bf = ml_dtypes.bfloat16

    in_maps = []
    for c in range(NCORES):
        b, hg = divmod(c, 2)
        heads = [HL * hg + i for i in range(HL)]

        def qk_w(W):
            Wsel = np.zeros((EP, D), np.float32)
            for i, g in enumerate(heads):
                Wsel[128 * i:128 * i + 128] = W[144 * g:144 * g + 128]
                Wsel[512 + 32 * i:512 + 32 * i + 16] = W[144 * g + 128:144 * g + 144]
            return np.ascontiguousarray(Wsel.T)

        wv_sel = np.concatenate([Wv[144 * g:144 * g + 144] for g in heads], 0)
        wo_sel = np.concatenate([Wo[:, 144 * g:144 * g + 144] for g in heads], 1)
        in_maps.append(
            {
                "xT": np.ascontiguousarray(x[b].T).astype(bf),
                "wqT": qk_w(Wq).astype(bf),
                "wkT": qk_w(Wk).astype(bf),
                "wvT": np.ascontiguousarray(wv_sel.T).astype(bf),
                "woT": np.ascontiguousarray(wo_sel.T).astype(bf),
                "cosTa": cosTa.astype(bf),
                "sinTa": sinTa.astype(bf),
                "cosTb": cosTb.astype(bf),
                "sinTb": sinTb.astype(bf),
                "identB": np.eye(128, dtype=bf),
            }
        )
    return in_maps


def kernel(x, cos, sin, Wq, Wk, Wv, Wo, _trace=False, _trace_kwargs=None):
    from concourse.bass_utils import run_bass_kernel_spmd

    nc = get_nc()
    in_maps = make_in_maps(x, cos, sin, Wq, Wk, Wv, Wo)
    res = run_bass_kernel_spmd(
        nc,
        in_maps,
        list(range(NCORES)),
        trace=_trace,
        **(_trace_kwargs or {}),
    )
    parts = [res.results[c]["out"] for c in range(NCORES)]
    outb = np.stack([parts[2 * b] + parts[2 * b + 1] for b in range(B)])
    if _trace:
        kernel.last_results = res
    return outb.astype(np.float32)


# revision 10
# speedup vs baseline: 1.5189x; 1.2240x over previous
"""Trainium2 Bass kernel for a fused multi-head attention block.

Reference computation (B=4, T=2048, D=1152, H=8, HD=144, full rotary):
    q,k,v = x@Wq.T, x@Wk.T, x@Wv.T   (per head)
    q,k   = rope(q, k, cos, sin)
    o     = softmax(q k^T / sqrt(HD)) v
    out   = o @ Wo.T
Sharding (8 cores): core c = (batch b = c//2, head-group hg = c%2).
Each core computes 4 heads of one batch and a partial output
out_part = o_local @ Wo[:, hg_cols].T ; host sums the two partials per batch.

v2 design (vs. the transpose-heavy v1):
  * q/k are projected DIRECTLY transposed: qT = Wsel^T-chunks (stationary)
    x xT (moving), so the scores layout [dim, T] needs no PE transposes.
  * rope runs in the transposed layout: the rotate-half partner lives at a
    partition offset, so a DMA SBUF->SBUF copy builds a partition-shifted
    replica qS with qS[e] = q[partner(e)]; then
    q_rot = q * cosT + qS * sinT_signed  (3 bf16 tensor_tensor ops, the
    rotate-half signs are folded into sinT_signed on the host).
  * per-head packing (same as v1): heads' dims 0..127 in four [128,T] tiles,
    dims 128..143 in a shared b-block tile at rows 32h..32h+16 (+16 zero pad),
    replicated to all four 32-row groups so the K=32 score-tail matmuls can
    run concurrently via tile_position.
  * scores S^T accumulate in a [128,1024] PSUM tile (two banks, two
    key-tiles per group) so each Exp activation covers 1024 elems/partition —
    halving ScalarE instruction overhead vs. [128,512] activations.
  * softmax denominator via ones-column appended to v (o_ps[:,144]).
  * phase C (oT transpose + final projection) runs fully in bf16 and all
    PSUM->SBUF evacuations go to Vector/Scalar (never the same engine that
    gates the PE), keeping the PE stream dense so HAM stays at K=8/8.
  * 16 zero matmuls at t=0 prime the PE activity monitor while the first
    DMAs land; a dummy Exp preloads the ACT table set.
"""

import numpy as np

B, T, D, H = 4, 2048, 1152, 8
HL = 4              # heads per core
HD = 144            # head dim
EP = 640            # packed q/k width: 4*128 + 128 (4x(16+16pad))
DV = HL * HD        # 576, v/o width
VW = HL * (HD + 1)  # 580, v + ones col
NT = T // 128       # 16 t-tiles
KC = D // 128       # 9 contraction chunks
SCALE = float(HD) ** -0.5
NCORES = 8

_NC_CACHE = {}


def _build(debug=False):
    import concourse.bacc as bacc
    import concourse.mybir as mybir
    from concourse.tile import TileContext

    dt = mybir.dt
    f32, bf16 = dt.float32, dt.bfloat16
    AF = mybir.ActivationFunctionType

    nc = bacc.Bacc(
        "TRN2",
        target_bir_lowering=False,
        debug=debug,
        enable_asserts=False,
        num_devices=NCORES,
    )

    xT = nc.declare_dram_parameter("xT", [D, T], bf16, isOutput=False)
    wqT = nc.declare_dram_parameter("wqT", [D, EP], bf16, isOutput=False)
    wkT = nc.declare_dram_parameter("wkT", [D, EP], bf16, isOutput=False)
    wvT = nc.declare_dram_parameter("wvT", [D, DV], bf16, isOutput=False)
    woT = nc.declare_dram_parameter("woT", [DV, D], bf16, isOutput=False)
    cosTa = nc.declare_dram_parameter("cosTa", [128, T], bf16, isOutput=False)
    sinTa = nc.declare_dram_parameter("sinTa", [128, T], bf16, isOutput=False)
    cosTb = nc.declare_dram_parameter("cosTb", [128, T], bf16, isOutput=False)
    sinTb = nc.declare_dram_parameter("sinTb", [128, T], bf16, isOutput=False)
    identB = nc.declare_dram_parameter("identB", [128, 128], bf16, isOutput=False)
    out = nc.declare_dram_parameter("out", [T, D], f32, isOutput=True)

    with TileContext(nc) as tc:
        with tc.tile_pool(name="persist", bufs=1) as P0:
            ident_bf = P0.tile([128, 128], bf16, name="ident_bf", tag="ident_bf")
            nc.sync.dma_start(ident_bf[:], identB[:])

            qTa = [P0.tile([128, T], bf16, name=f"qTa{h}", tag=f"qTa{h}")
                   for h in range(HL)]
            kTa = [P0.tile([128, T], bf16, name=f"kTa{h}", tag=f"kTa{h}")
                   for h in range(HL)]
            qTBr = [P0.tile([128, T], bf16, name=f"qTBr{h}", tag=f"qTBr{h}")
                    for h in range(HL)]
            kTBr = [P0.tile([128, T], bf16, name=f"kTBr{h}", tag=f"kTBr{h}")
                    for h in range(HL)]
            vt = [P0.tile([128, VW], bf16, name=f"v{t}", tag=f"v{t}")
                  for t in range(NT)]

            # ------------- Phase A: transposed projections + rope ----------
            with (
                tc.tile_pool(name="pa", bufs=1) as pa,
                tc.tile_pool(name="paps", bufs=1, space="PSUM") as paps,
            ):
                # PE warm-up: zero matmuls keep the activity monitor busy
                # while the first weight/x DMAs land.
                wup = pa.tile([128, 512], bf16, name="wup", tag="wup")
                nc.vector.memset(wup[:], 0.0)
                for _ in range(16):
                    wps = paps.tile([128, 512], f32, name="wps", tag="wps", bufs=2)
                    nc.tensor.matmul(wps[:], wup[:, 0:128], wup[:],
                                     start=True, stop=True)
                # preload the exp table set early (one-time ~2.7us)
                dumm = pa.tile([128, 8], f32, name="dumm", tag="dumm")
                nc.scalar.activation(dumm[:], wup[:, 0:8], AF.Exp)

                xt = [pa.tile([128, T], bf16, name=f"xt{k}", tag=f"xt{k}")
                      for k in range(KC)]
                cos_a = pa.tile([128, T], bf16, name="cos_a", tag="cos_a")
                sin_a = pa.tile([128, T], bf16, name="sin_a", tag="sin_a")
                cos_b = pa.tile([128, T], bf16, name="cos_b", tag="cos_b")
                sin_b = pa.tile([128, T], bf16, name="sin_b", tag="sin_b")

                def qk_phase(wdram, dstA, dstBr, first=False):
                    wsb = []
                    for k in range(KC):
                        wt = pa.tile([128, EP], bf16, name=f"w{k}", tag=f"W{k}")
                        # b-cols first: the B block is projected first
                        nc.sync.dma_start(
                            wt[:, 512:EP], wdram[k * 128:(k + 1) * 128, 512:EP])
                        nc.sync.dma_start(
                            wt[:, 0:512], wdram[k * 128:(k + 1) * 128, 0:512])
                        wsb.append(wt)
                        if first:
                            # x chunk right after its weight chunk: the k-outer
                            # matmul order consumes (w[k], xt[k]) pairs in k
                            # order, so chains can start as soon as pair 0 lands
                            for j in range(4):
                                nc.sync.dma_start(
                                    xt[k][:, j * 512:(j + 1) * 512],
                                    xT[k * 128:(k + 1) * 128, j * 512:(j + 1) * 512])

                    def proj_block(cols, dst):
                        # dst (bf16 SBUF) <- (wsb[:, cols]).T @ xt
                        # k-outer: one stationary load feeds all 4 t-chains
                        pss = [paps.tile([128, 512], f32, name=f"pps{tg}",
                                         tag=f"projps{tg}", bufs=1)
                               for tg in range(4)]
                        for k in range(KC):
                            for tg in range(4):
                                nc.tensor.matmul(
                                    pss[tg][:], wsb[k][:, cols],
                                    xt[k][:, tg * 512:(tg + 1) * 512],
                                    start=(k == 0), stop=(k == KC - 1))
                        for tg in range(4):
                            nc.scalar.copy(dst[:, tg * 512:(tg + 1) * 512],
                                           pss[tg][:])

                    rawB = pa.tile([128, T], bf16, name="rawB", tag="rawB")
                    proj_block(slice(512, EP), rawB)
                    if first:
                        # trig loads delayed behind the B-block evac on the
                        # scalar FIFO so they don't steal HBM bandwidth from
                        # the critical first w/x loads
                        nc.scalar.dma_start(cos_a[:], cosTa[:])
                        nc.scalar.dma_start(sin_a[:], sinTa[:])
                        nc.scalar.dma_start(cos_b[:], cosTb[:])
                        nc.scalar.dma_start(sin_b[:], sinTb[:])
                    qSB = pa.tile([128, T], bf16, name="qSB", tag="qSB")
                    nc.gpsimd.memset(qSB[:], 0.0)

                    for h in range(HL):
                        rawA = pa.tile([128, T], bf16, name="rawA",
                                       tag="rawA", bufs=3)
                        proj_block(slice(h * 128, (h + 1) * 128), rawA)
                        # partition-shifted replica qS[e] = raw[partner(e)]
                        qS = pa.tile([128, T], bf16, name="qS", tag="qS", bufs=2)
                        nc.gpsimd.dma_start(qS[0:56, :], rawA[72:128, :])
                        nc.gpsimd.dma_start(qS[56:72, :], rawB[32 * h:32 * h + 16, :])
                        nc.gpsimd.dma_start(qS[72:128, :], rawA[0:56, :])
                        nc.gpsimd.dma_start(qSB[32 * h:32 * h + 16, :], rawA[56:72, :])
                        m1 = pa.tile([128, T], bf16, name="m1", tag="m1", bufs=2)
                        m2 = pa.tile([128, T], bf16, name="m2", tag="m2", bufs=2)
                        nc.vector.tensor_mul(m1[:], qS[:], sin_a[:])
                        nc.vector.tensor_mul(m2[:], rawA[:], cos_a[:])
                        nc.vector.tensor_add(dstA[h][:], m1[:], m2[:])

                    # b-block rope + 4x row-group replication
                    mB1 = pa.tile([128, T], bf16, name="mB1", tag="m1", bufs=2)
                    mB2 = pa.tile([128, T], bf16, name="mB2", tag="m2", bufs=2)
                    qTB = pa.tile([128, T], bf16, name="qTB", tag="qTB")
                    nc.vector.tensor_mul(mB1[:], qSB[:], sin_b[:])
                    nc.vector.tensor_mul(mB2[:], rawB[:], cos_b[:])
                    nc.vector.tensor_add(qTB[:], mB1[:], mB2[:])
                    for h in range(HL):
                        for j in range(4):
                            nc.gpsimd.dma_start(
                                dstBr[h][32 * j:32 * j + 32, :],
                                qTB[32 * h:32 * h + 32, :])

                # q first, then k, then v: each phase's b-block rope +
                # replication tail is hidden under the next phase's matmuls,
                # so the first attention group starts with all deps ready.
                qk_phase(wqT, qTa, qTBr, first=True)
                qk_phase(wkT, kTa, kTBr)

                # ---- v projection (natural [t, e] layout) ----
                wv_sb = []
                for k in range(KC):
                    wt = pa.tile([128, DV], bf16, name=f"wv{k}", tag=f"W{k}")
                    nc.sync.dma_start(wt[:], wvT[k * 128:(k + 1) * 128, :])
                    wv_sb.append(wt)
                for n in range(NT):
                    pg = 2 * (n % 2)   # alternate tag pairs = double buffering
                    ps0 = paps.tile([128, 288], f32, name="ps0",
                                    tag=f"projps{pg}", bufs=1)
                    ps1 = paps.tile([128, 288], f32, name="ps1",
                                    tag=f"projps{pg + 1}", bufs=1)
                    for k in range(KC):
                        lhs = xt[k][:, n * 128:(n + 1) * 128]
                        nc.tensor.matmul(ps0[:], lhs, wv_sb[k][:, 0:288],
                                         start=(k == 0), stop=(k == KC - 1))
                        nc.tensor.matmul(ps1[:], lhs, wv_sb[k][:, 288:DV],
                                         start=(k == 0), stop=(k == KC - 1))
                    v3 = vt[n].rearrange("p (h e) -> p h e", h=HL)
                    nc.scalar.copy(v3[:, 0:2, 0:HD],
                                   ps0.rearrange("p (h e) -> p h e", h=2))
                    nc.scalar.copy(v3[:, 2:4, 0:HD],
                                   ps1.rearrange("p (h e) -> p h e", h=2))
                    nc.vector.memset(v3[:, :, HD:HD + 1], 1.0)

            # ------------- Phase B: attention ------------------------------
            with tc.tile_pool(name="pb", bufs=1) as pb:
                ot = [pb.tile([128, DV], bf16, name=f"o{t}", tag=f"o{t}")
                      for t in range(NT)]
                with tc.tile_pool(name="pbps", bufs=1, space="PSUM") as pbps:
                    for qb in range(4):
                        for h in range(HL):
                            o_ps3 = pbps.tile([128, 3 * (HD + 1)], f32,
                                              name="o_ps3", tag="o3", bufs=1)
                            o_ps1 = pbps.tile([128, HD + 1], f32,
                                              name="o_ps1", tag="o1", bufs=1)
                            o_ps = [
                                o_ps3[:, 0:HD + 1],
                                o_ps3[:, HD + 1:2 * (HD + 1)],
                                o_ps3[:, 2 * (HD + 1):3 * (HD + 1)],
                                o_ps1[:],
                            ]

                            # key-tile groups of (3,3,3,3,2,2): one [128,1536]
                            # score-PSUM tile (3 banks) per group -> one Exp
                            # covers up to 1536 elems/partition
                            GRPS = [(0, 3), (3, 3), (6, 3), (9, 3),
                                    (12, 2), (14, 2)]

                            def s_exp(g):
                                kt0, gn = GRPS[g]
                                sps = pbps.tile([128, 1536], f32, name="sps",
                                                tag="sc", bufs=2)
                                # K=32 b-block tails first (start=True clears
                                # the bank), so the K=128 mains run
                                # back-to-back with stop=True
                                for j in range(gn):
                                    kt = kt0 + j
                                    rg = kt % 4
                                    nc.tensor.matmul(
                                        sps[:, j * 512:(j + 1) * 512],
                                        kTBr[h][32 * rg:32 * rg + 32,
                                                kt * 128:(kt + 1) * 128],
                                        qTBr[h][32 * rg:32 * rg + 32,
                                                qb * 512:(qb + 1) * 512],
                                        start=True, stop=False,
                                        tile_position=(32 * rg, 0))
                                for j in range(gn):
                                    kt = kt0 + j
                                    nc.tensor.matmul(
                                        sps[:, j * 512:(j + 1) * 512],
                                        kTa[h][:, kt * 128:(kt + 1) * 128],
                                        qTa[h][:, qb * 512:(qb + 1) * 512],
                                        start=False, stop=True)
                                E = pb.tile([128, 1536], bf16, name="E",
                                            tag="E", bufs=4)
                                nc.scalar.activation(E[:, 0:gn * 512],
                                                     sps[:, 0:gn * 512],
                                                     AF.Exp, scale=SCALE)
                                return E

                            def pv(g, E):
                                kt0, gn = GRPS[g]
                                for j in range(gn):
                                    kt = kt0 + j
                                    for qt in range(4):
                                        if qt < 3:
                                            st = kt == 0 and qt == 0
                                            sp = kt == NT - 1 and qt == 2
                                        else:
                                            st = kt == 0
                                            sp = kt == NT - 1
                                        nc.tensor.matmul(
                                            o_ps[qt][:],
                                            E[:, j * 512 + qt * 128:
                                              j * 512 + (qt + 1) * 128],
                                            vt[kt][:, (HD + 1) * h:
                                                   (HD + 1) * (h + 1)],
                                            start=st, stop=sp)

                            ngrp = len(GRPS)
                            Ep = s_exp(0)
                            for g in range(ngrp):
                                En = s_exp(g + 1) if g + 1 < ngrp else None
                                pv(g, Ep)
                                Ep = En
                            for qt in range(4):
                                t = qb * 4 + qt
                                r = pb.tile([128, 1], f32, name="r", tag="r",
                                            bufs=4)
                                nc.vector.reciprocal(r[:], o_ps[qt][:, HD:HD + 1])
                                nc.vector.tensor_scalar_mul(
                                    ot[t][:, HD * h:HD * (h + 1)],
                                    o_ps[qt][:, 0:HD], r[:])

                # ------------- Phase C: o^T + final projection -------------
                oTa = [pb.tile([128, T], bf16, name=f"oTa{j}", tag=f"oTa{j}")
                       for j in range(4)]
                oTb = pb.tile([64, T], bf16, name="oTb", tag="oTb")
                wo_sb = []
                for k in range(5):
                    rows = 128 if k < 4 else 64
                    wot = pb.tile([128, D], bf16, name=f"wo{k}", tag=f"wo{k}")
                    nc.sync.dma_start(wot[0:rows, :], woT[k * 128:k * 128 + rows, :])
                    wo_sb.append(wot)
                with tc.tile_pool(name="pcps", bufs=1, space="PSUM") as pcps:

                    def o_transp(t):
                        for j in range(4):
                            tp = pcps.tile([128, 128], bf16, name="tpo",
                                           tag="otp", bufs=4)
                            nc.tensor.transpose(
                                tp[:], ot[t][:, 128 * j:128 * (j + 1)],
                                ident_bf[:])
                            nc.scalar.copy(oTa[j][:, t * 128:(t + 1) * 128], tp[:])
                        tpb = pcps.tile([64, 128], bf16, name="tpb",
                                        tag="otp", bufs=4)
                        nc.tensor.transpose(tpb[:], ot[t][:, 512:DV], ident_bf[:])
                        nc.scalar.copy(oTb[:, t * 128:(t + 1) * 128], tpb[:])

                    def final(t):
                        for j3 in range(3):
                            fps = pcps.tile([128, 384], f32, name="fps",
                                            tag="f", bufs=3)
                            for k in range(5):
                                lhs = (oTa[k][:, t * 128:(t + 1) * 128]
                                       if k < 4
                                       else oTb[:, t * 128:(t + 1) * 128])
                                nc.tensor.matmul(
                                    fps[:], lhs,
                                    wo_sb[k][0:(128 if k < 4 else 64),
                                             384 * j3:384 * (j3 + 1)],
                                    start=(k == 0), stop=(k == 4))
                            fout = pb.tile([128, 384], f32, name="fout",
                                           tag="fout", bufs=4)
                            nc.vector.tensor_copy(fout[:], fps[:])
                            nc.sync.dma_start(
                                out[t * 128:(t + 1) * 128,
                                    384 * j3:384 * (j3 + 1)], fout[:])

                    o_transp(0)
                    for t in range(NT):
                        if t + 1 < NT:
                            o_transp(t + 1)
                        final(t)

    nc.compile()
    return nc


def get_nc(debug=False):
    key = bool(debug)
    if key not in _NC_CACHE:
        _NC_CACHE[key] = _build(debug)
    return _NC_CACHE[key]


def make_in_maps(x, cos, sin, Wq, Wk, Wv, Wo):
    import ml_dtypes

    x = np.asarray(x, np.float32)
    cos = np.asarray(cos, np.float32)
    sin = np.asarray(sin, np.float32)
    Wq, Wk, Wv, Wo = (np.asarray(w, np.float32) for w in (Wq, Wk, Wv, Wo))

    # transposed trig tables with the rotate-half signs folded in:
    # out[e] = raw[e]*cos[e] + sgn(e)*raw[partner(e)]*sin[e]
    cosT = np.ascontiguousarray(cos.T)   # [144, T]
    sinT = np.ascontiguousarray(sin.T)
    sgn = np.ones((128, 1), np.float32)
    sgn[:72] = -1.0
    cosTa = cosT[0:128]
    sinTa = sinT[0:128] * sgn
    cosTb = np.zeros((128, T), np.float32)
    sinTb = np.zeros((128, T), np.float32)
    for hh in range(HL):
        cosTb[32 * hh:32 * hh + 16] = cosT[128:144]
        sinTb[32 * hh:32 * hh + 16] = sinT[128:144]
    bf = ml_dtypes.bfloat16

    in_maps = []
    for c in range(NCORES):
        b, hg = divmod(c, 2)
        heads = [HL * hg + i for i in range(HL)]

        def qk_w(W):
            Wsel = np.zeros((EP, D), np.float32)
            for i, g in enumerate(heads):
                Wsel[128 * i:128 * i + 128] = W[144 * g:144 * g + 128]
                Wsel[512 + 32 * i:512 + 32 * i + 16] = W[144 * g + 128:144 * g + 144]
            return np.ascontiguousarray(Wsel.T)

        wv_sel = np.concatenate([Wv[144 * g:144 * g + 144] for g in heads], 0)
        wo_sel = np.concatenate([Wo[:, 144 * g:144 * g + 144] for g in heads], 1)
        in_maps.append(
            {
                "xT": np.ascontiguousarray(x[b].T).astype(bf),
                "wqT": qk_w(Wq).astype(bf),
                "wkT": qk_w(Wk).astype(bf),
                "wvT": np.ascontiguousarray(wv_sel.T).astype(bf),
                "woT": np.ascontiguousarray(wo_sel.T).astype(bf),
                "cosTa": cosTa.astype(bf),
                "sinTa": sinTa.astype(bf),
                "cosTb": cosTb.astype(bf),
                "sinTb": sinTb.astype(bf),
                "identB": np.eye(128, dtype=bf),
            }
        )
    return in_maps


def kernel(x, cos, sin, Wq, Wk, Wv, Wo, _trace=False, _trace_kwargs=None):
    from concourse.bass_utils import run_bass_kernel_spmd

    nc = get_nc()
    in_maps = make_in_maps(x, cos, sin, Wq, Wk, Wv, Wo)
    res = run_bass_kernel_spmd(
        nc,
        in_maps,
        list(range(NCORES)),
        trace=_trace,
        **(_trace_kwargs or {}),
    )
    parts = [res.results[c]["out"] for c in range(NCORES)]
    outb = np.stack([parts[2 * b] + parts[2 * b + 1] for b in range(B)])
    if _trace:
        kernel.last_results = res
    return outb.astype(np.float32)
